# revision 24
# baseline (speedup 1.0000x reference)
"""Multi-head attention + residual + LayerNorm Trainium2 kernel.

Shapes (hardcoded from the problem spec):
  input_Q/K/V: [B=8, S=1024, D=1024] f32, attn_mask: [8, 1024, 1024] f32
  W_Q/W_K/W_V: [1024, 1024] f32, W_O: [1024, 1024] f32
  outputs: out [8, 1024, 1024] f32, attn [8, 16, 1024, 1024] f32

Sharding: batch-parallel - core b handles batch element b (weights replicated,
no collectives). Inside each core:
  phase 1: X^T via PE transpose; Q^T/K^T/V^T = W^T X^T with W streamed in
           stationary stripes; V natural recovered from V^T by PE transpose,
           with a ones column interleaved per head so attention row sums
           fall out of the attn@V matmul for free
  phase 2: per head, scores twice (natural [sq,sk] for softmax+attn output,
           transposed [sk,sq] for attn@V), exp on ACT, normalize on DVE
  phase 3: out = LayerNorm(ctx @ W_O + X_q)
All matmuls run in fp32r (4x faster than fp32 on trn2, ~1e-4 rel err).
"""

import contextlib

import numpy as np

import concourse.bass as bass
import concourse.mybir as mybir
import concourse.tile as tile
from concourse.bass_utils import run_bass_kernel_spmd
from bass_rust import ScopedClock

P = 128
S = 1024
D = 1024
H = 16
DK = 64
SB = S // P       # 8 seq blocks of 128
KB = D // P       # 8 contraction blocks of 128
HB = H * DK // P  # 8 head-dim blocks of 128 (2 heads per block)
F = 512           # matmul free-dim chunk (one PSUM bank of f32)
FB = S // F       # 2
VW = DK + 1       # V columns per head incl. ones column
LN_EPS = 1e-5

f32 = mybir.dt.float32
f32r = mybir.dt.float32r
AF = mybir.ActivationFunctionType


class TC(tile.TileContext):
    """TileContext whose exit drain splits sem waits one-per-instruction.

    The walrus build in this container rejects TPB_CTRL instructions with
    more than one sync wait; the stock exit drain aggregates one wait per
    engine/DMA-queue used by the kernel.
    """

    def _drain_and_barrier(self, tick_clock, wait_clock):
        probe = self.nc.sync.nop()
        wait_clock.add_sem_waits(
            probe.ins, ScopedClock({None: tick_clock.global_clock})
        )
        si = probe.ins.sync_info
        waits = list(si.on_wait) if si is not None else []
        if len(waits) > 1:
            probe.ins.sync_info = mybir.SyncInfo(on_wait=[waits[0]], on_update=[])
            for wt in waits[1:]:
                n = self.nc.sync.nop()
                n.ins.sync_info = mybir.SyncInfo(on_wait=[wt], on_update=[])
        self.nc.sync.drain()
        self.nc.all_engine_barrier()
        popped = self.nc._tile_sem_poison_stack.pop()
        assert popped is self._sem_poison
        self.nc.clear_and_free_semaphores(list(self.sems.allocated().values()))
        self.nc.all_engine_barrier()


def _phase1(nc, tc, ident, QT, KT, Vn, VT, xq, xk, xv, wq, wk, wv):
    """Projections. Produces QT/KT/VT [hd%128, hdo, s] (f32r, transposed) and
    Vn [s%128, so, h*65+c] (V natural with ones columns)."""
    with tc.tile_pool(name="xstage", bufs=2) as xstage, \
         tc.tile_pool(name="xT", bufs=1) as xTp, \
         tc.tile_pool(name="wstage", bufs=2) as wstage, \
         tc.tile_pool(name="ps_tr", bufs=4, space="PSUM") as ps_tr, \
         tc.tile_pool(name="ps_mm", bufs=4, space="PSUM") as ps_mm:

        # ones columns of V (written once; transpose copies skip them)
        ones_view = Vn[:, :, :].rearrange(
            "p so (h c) -> p so h c", c=VW
        )[:, :, :, DK : DK + 1]
        ones_f = xstage.tile([P, SB, H, 1], f32, tag="onesf")
        nc.vector.memset(ones_f, 1.0)
        nc.gpsimd.tensor_copy(out=ones_view, in_=ones_f)

        for name, X, W, dst in (("q", xq, wq, QT), ("k", xk, wk, KT),
                                ("v", xv, wv, VT)):
            # stage X, round to f32r, transpose into xT [d%128, do, s]
            xT = xTp.tile([P, KB, S], f32r, tag="xT")
            for so in range(SB):
                xs = xstage.tile([P, D], f32, tag="xs")
                nc.sync.dma_start(out=xs, in_=X[so * P : (so + 1) * P, :])
                xr = xstage.tile([P, D], f32r, tag="xr")
                nc.gpsimd.tensor_copy(out=xr, in_=xs)
                for do in range(KB):
                    pt = ps_tr.tile([P, P], f32r, tag="tr")
                    nc.tensor.transpose(pt, xr[:, do * P : (do + 1) * P], ident)
                    nc.vector.tensor_copy(
                        out=xT[:, do, so * P : (so + 1) * P], in_=pt
                    )

            # dst^T[hd, s] = W[k, hd]^T @ X^T[k, s], W streamed per ho-stripe
            for ho in range(HB):
                ws = wstage.tile([P, KB, P], f32, tag="ws")
                nc.sync.dma_start(
                    out=ws,
                    in_=W.rearrange("(ko ki) n -> ki ko n", ki=P)[
                        :, :, ho * P : (ho + 1) * P
                    ],
                )
                wr = wstage.tile([P, KB, P], f32r, tag="wr")
                nc.gpsimd.tensor_copy(out=wr, in_=ws)
                pms = [ps_mm.tile([P, F], f32, tag="mm", name=f"pm{i}") for i in range(FB)]
                for ko in range(KB):
                    for fb in range(FB):
                        nc.tensor.matmul(
                            pms[fb],
                            lhsT=wr[:, ko, :],
                            rhs=xT[:, ko, fb * F : (fb + 1) * F],
                            start=(ko == 0),
                            stop=(ko == KB - 1),
                        )
                for fb in range(FB):
                    nc.vector.tensor_copy(
                        out=dst[:, ho, fb * F : (fb + 1) * F], in_=pms[fb]
                    )

        # V natural from V^T via PE transpose, into 65-wide per-head slots
        for so in range(SB):
            for ho in range(HB):
                pt = ps_tr.tile([P, P], f32r, tag="tr")
                nc.tensor.transpose(pt, VT[:, ho, so * P : (so + 1) * P], ident)
                dstv = Vn[
                    :, so, ho * 2 * VW : (ho * 2 + 2) * VW
                ].rearrange("p (h c) -> p h c", c=VW)[:, :, :DK]
                nc.vector.tensor_copy(
                    out=dstv, in_=pt.rearrange("p (h c) -> p h c", c=DK)
                )


def _phase2(nc, tc, QT, KT, Vn, ctxT, attn, use_mask, madd, maddT, ones32, ident32):
    with tc.tile_pool(name="expnat", bufs=3) as expp, \
         tc.tile_pool(name="attnout", bufs=3) as attp, \
         tc.tile_pool(name="expT", bufs=3) as expTp, \
         tc.tile_pool(name="stats", bufs=8) as statp, \
         tc.tile_pool(name="rpad", bufs=1) as rpadp, \
         (tc.tile_pool(name="maskst", bufs=2) if use_mask
          else contextlib.nullcontext()) as maskp, \
         tc.tile_pool(name="ps_nat", bufs=1, space="PSUM") as ps_nat, \
         tc.tile_pool(name="ps_T", bufs=2, space="PSUM") as ps_T, \
         tc.tile_pool(name="ps_ctx", bufs=1, space="PSUM") as ps_ctx:

        # 32-row staging for 1/sums: row 0 live, rows 1-31 stay zero
        rTpad_f = rpadp.tile([32, S], f32, tag="rTpf")
        nc.vector.memset(rTpad_f, 0.0)
        rTpad = rpadp.tile([32, S], f32r, tag="rTp")
        nc.gpsimd.tensor_copy(out=rTpad, in_=rTpad_f)

        for h in range(H):
            ho, hi = h // 2, (h % 2) * DK
            qt_h = QT[hi : hi + DK, ho, :]
            kt_h = KT[hi : hi + DK, ho, :]

            # --- transposed path first: ctx^T = V^T @ softmax^T, and the
            # per-query row sums (ones row of the AV matmul) ---
            pcs = [ps_ctx.tile([VW, F], f32, tag=f"ctx{fb}", name=f"pc{fb}") for fb in range(FB)]
            for skb in range(SB):
                pt = ps_T.tile([P, S], f32, tag="sT")
                for fb in range(FB):
                    nc.tensor.matmul(
                        pt[:, fb * F : (fb + 1) * F],
                        lhsT=kt_h[:, skb * P : (skb + 1) * P],
                        rhs=qt_h[:, fb * F : (fb + 1) * F],
                        start=True,
                        stop=True,
                    )
                if use_mask:
                    mtT = maskp.tile([P, S], f32, tag="mT")
                    nc.sync.dma_start(
                        out=mtT, in_=maddT[skb * P : (skb + 1) * P, :]
                    )
                    mTs = expTp.tile([P, S], f32, tag="mTs")
                    nc.vector.tensor_add(out=mTs, in0=pt, in1=mtT)
                    eT_src = mTs
                else:
                    eT_src = pt
                eT = expTp.tile([P, S], f32r, tag="eT")
                nc.scalar.activation(out=eT, in_=eT_src, func=AF.Exp, scale=0.125)
                for fb in range(FB):
                    nc.tensor.matmul(
                        pcs[fb],
                        lhsT=Vn[:, skb, h * VW : (h + 1) * VW],
                        rhs=eT[:, fb * F : (fb + 1) * F],
                        start=(skb == 0),
                        stop=(skb == SB - 1),
                    )
            # normalize ctx^T by the ones-row sums. Engines cannot
            # partition-broadcast, so write 1/sum into row 0 of a 32-row
            # tile (rows 1-31 zero) and use K=32 matmuls to broadcast it
            # along partitions and to flip it into sq-partition layout.
            for fb in range(FB):
                pc = pcs[fb]
                with nc.allow_low_precision(reason="f32r is f32-width"):
                    nc.vector.reciprocal(
                        out=rTpad[0:1, fb * F : (fb + 1) * F],
                        in_=pc[DK : DK + 1, :],
                    )
            for fb in range(FB):
                pc = pcs[fb]
                rTb = ps_T.tile([P, S], f32, tag="sT")
                nc.tensor.matmul(
                    rTb[:DK, :F],
                    lhsT=ones32,
                    rhs=rTpad[:, fb * F : (fb + 1) * F],
                    start=True,
                    stop=True,
                )
                rTbs = expTp.tile([DK, F], f32, tag="rTbs")
                nc.vector.tensor_copy(out=rTbs, in_=rTb[:DK, :F])
                nc.vector.tensor_tensor(
                    out=ctxT[hi : hi + DK, ho, fb * F : (fb + 1) * F],
                    in0=pc[:DK, :],
                    in1=rTbs,
                    op=mybir.AluOpType.mult,
                )
            # flip 1/sum into [sq-partition, so] layout: each 128-chunk of
            # rTpad row 0 becomes one psum column, then ln() so exp can
            # normalize via its bias: attn = exp(s/8 + ln(1/sum)).
            pcol = ps_ctx.tile([P, SB * 32], f32r, tag="ctx0", name="pcol")
            for so in range(SB):
                nc.tensor.transpose(
                    pcol[:, so * 32 : (so + 1) * 32],
                    rTpad[:, so * P : (so + 1) * P],
                    ident32,
                )
            lnb_raw = statp.tile([P, SB], f32, tag="lnr")
            nc.vector.tensor_copy(
                out=lnb_raw,
                in_=pcol.rearrange("p (so c) -> p so c", c=32)[:, :, 0],
            )
            lnb = statp.tile([P, SB], f32, tag="lnb")
            nc.scalar.activation(out=lnb, in_=lnb_raw, func=AF.Ln)

            # --- natural path: attn output straight from the exp ---
            for so in range(SB):
                ps = ps_nat.tile([P, S], f32, tag="nat")
                for fb in range(FB):
                    nc.tensor.matmul(
                        ps[:, fb * F : (fb + 1) * F],
                        lhsT=qt_h[:, so * P : (so + 1) * P],
                        rhs=kt_h[:, fb * F : (fb + 1) * F],
                        start=True,
                        stop=True,
                    )
                if use_mask:
                    mt = maskp.tile([P, S], f32, tag="m")
                    nc.sync.dma_start(out=mt, in_=madd[so * P : (so + 1) * P, :])
                    msum = expp.tile([P, S], f32, tag="msum")
                    nc.vector.tensor_add(out=msum, in0=ps, in1=mt)
                    exp_src = msum
                else:
                    exp_src = ps
                at_ = attp.tile([P, S], f32, tag="a")
                nc.scalar.activation(
                    out=at_, in_=exp_src, func=AF.Exp, scale=0.125,
                    bias=lnb[:, so : so + 1],
                )
                nc.sync.dma_start(out=attn[h, so * P : (so + 1) * P, :], in_=at_)


def _phase3(nc, tc, singles, ctxT, wo, xq, out):
    with tc.tile_pool(name="wostage", bufs=2) as wostage, \
         tc.tile_pool(name="wopool", bufs=2) as wop, \
         tc.tile_pool(name="xqres", bufs=2) as xqp, \
         tc.tile_pool(name="xout", bufs=3) as xop, \
         tc.tile_pool(name="lnstat", bufs=6) as lnp, \
         tc.tile_pool(name="ps_o", bufs=4, space="PSUM") as ps_o:

        eps_t = singles.tile([P, 1], f32)
        nc.vector.memset(eps_t, LN_EPS)

        # W_O: stage per (fb, ho) block, round into 2 resident f32r stripes
        wos = []
        for fb in range(FB):
            wsr = wop.tile([P, HB, F], f32r, tag="wor", name=f"wsr{fb}")
            for ho in range(HB):
                wsf = wostage.tile([P, F], f32, tag="wos")
                nc.sync.dma_start(
                    out=wsf,
                    in_=wo[ho * P : (ho + 1) * P, fb * F : (fb + 1) * F],
                )
                nc.gpsimd.tensor_copy(out=wsr[:, ho, :], in_=wsf)
            wos.append(wsr)

        for so in range(SB):
            xqs = xqp.tile([P, D], f32, tag="xq")
            nc.sync.dma_start(out=xqs, in_=xq[so * P : (so + 1) * P, :])
            xx = xop.tile([P, D], f32, tag="xx")
            for fb in range(FB):
                po = ps_o.tile([P, F], f32, tag="o")
                for ho in range(HB):
                    nc.tensor.matmul(
                        po,
                        lhsT=ctxT[:, ho, so * P : (so + 1) * P],
                        rhs=wos[fb][:, ho, :],
                        start=(ho == 0),
                        stop=(ho == HB - 1),
                    )
                nc.vector.tensor_add(
                    out=xx[:, fb * F : (fb + 1) * F],
                    in0=po,
                    in1=xqs[:, fb * F : (fb + 1) * F],
                )
            # LayerNorm over D (free dim)
            stats = lnp.tile([P, 2, 6], f32, tag="bs")
            for half in range(2):
                nc.vector.bn_stats(
                    out=stats[:, half, :], in_=xx[:, half * F : (half + 1) * F]
                )
            mv = lnp.tile([P, 2], f32, tag="mv")
            nc.vector.bn_aggr(out=mv, in_=stats)
            rstd = lnp.tile([P, 1], f32, tag="rstd")
            nc.scalar.activation(out=rstd, in_=mv[:, 1:2], func=AF.Sqrt, bias=eps_t)
            nc.vector.reciprocal(out=rstd, in_=rstd)
            ot = xop.tile([P, D], f32, tag="ot")
            nc.vector.tensor_scalar(
                out=ot,
                in0=xx,
                scalar1=mv[:, 0:1],
                scalar2=rstd,
                op0=mybir.AluOpType.subtract,
                op1=mybir.AluOpType.mult,
            )
            nc.sync.dma_start(out=out[so * P : (so + 1) * P, :], in_=ot)


def _legalize_waits(nc):
    """Split multi-wait instructions: this walrus build caps sync waits at 1
    per regular instruction (2 per EventSemaphore). Hoist extra waits onto
    EventSemaphore instructions inserted just before, on the same engine."""
    for fn in nc.m.functions:
        for bb in fn.blocks:
            insts = bb.instructions
            out = []
            changed = False
            for inst in insts:
                si = inst.sync_info
                waits = list(si.on_wait) if si is not None else []
                if len(waits) > 1:
                    changed = True
                    extra = waits[:-1]
                    for i in range(0, len(extra), 2):
                        ev = mybir.InstEventSemaphore(
                            name=nc.get_next_instruction_name(),
                            engine=inst.engine,
                            ins=[],
                            outs=[],
                            sync_info=mybir.SyncInfo(
                                on_wait=extra[i : i + 2], on_update=[]
                            ),
                        )
                        nc.register_instruction(ev, overwrite=True)
                        out.append(ev)
                    inst.sync_info = mybir.SyncInfo(
                        on_wait=[waits[-1]], on_update=list(si.on_update)
                    )
                out.append(inst)
            if changed:
                bb.instructions = out


def build_kernel(use_mask: bool):
    nc = bass.Bass()

    xq = nc.declare_dram_parameter("input_Q", [S, D], f32, isOutput=False)
    xk = nc.declare_dram_parameter("input_K", [S, D], f32, isOutput=False)
    xv = nc.declare_dram_parameter("input_V", [S, D], f32, isOutput=False)
    wq = nc.declare_dram_parameter("W_Q", [D, H * DK], f32, isOutput=False)
    wk = nc.declare_dram_parameter("W_K", [D, H * DK], f32, isOutput=False)
    wv = nc.declare_dram_parameter("W_V", [D, H * DK], f32, isOutput=False)
    wo = nc.declare_dram_parameter("W_O", [H * DK, D], f32, isOutput=False)
    madd = maddT = None
    if use_mask:
        # additive masks pre-scaled by 8 on host: 8e9*(mask-1), natural + transposed
        madd = nc.declare_dram_parameter("madd8", [S, S], f32, isOutput=False)
        maddT = nc.declare_dram_parameter("madd8T", [S, S], f32, isOutput=False)
    out = nc.declare_dram_parameter("out", [S, D], f32, isOutput=True)
    attn = nc.declare_dram_parameter("attn", [H, S, S], f32, isOutput=True)

    with TC(nc) as tc:
        with tc.tile_pool(name="singles", bufs=1) as singles, \
             tc.tile_pool(name="bigpool", bufs=1) as bigp:
            from concourse.masks import make_identity

            ident_f = singles.tile([P, P], f32)
            make_identity(nc, ident_f)
            ident = singles.tile([P, P], f32r)
            nc.gpsimd.tensor_copy(out=ident, in_=ident_f)
            ones32f = singles.tile([32, DK], f32)
            nc.vector.memset(ones32f, 1.0)
            ones32 = singles.tile([32, DK], f32r)
            nc.gpsimd.tensor_copy(out=ones32, in_=ones32f)

            QT = bigp.tile([P, HB, S], f32r, tag="qT")  # [hd%128, hdo, s]
            KT = bigp.tile([P, HB, S], f32r, tag="kT")
            Vn = bigp.tile([P, SB, H * VW], f32r, tag="v")
            # VT shares the "big" slot with ctxT (sequential lifetimes)
            VT = bigp.tile([P, HB, S], f32r, tag="big")
            _phase1(nc, tc, ident, QT, KT, Vn, VT, xq, xk, xv, wq, wk, wv)

            ctxT = bigp.tile([P, HB, S], f32r, tag="big")
            _phase2(nc, tc, QT, KT, Vn, ctxT, attn, use_mask, madd, maddT, ones32, ident[:32, :32])
            _phase3(nc, tc, singles, ctxT, wo, xq, out)

    _legalize_waits(nc)
    return nc


_CACHE = {}


def _get_nc(use_mask: bool):
    if use_mask not in _CACHE:
        _CACHE[use_mask] = build_kernel(use_mask)
    return _CACHE[use_mask]


def kernel(input_Q, input_K, input_V, attn_mask, W_Q, W_K, W_V, W_O):
    B = input_Q.shape[0]
    assert B == 8
    use_mask = not bool(np.all(attn_mask == 1.0))
    nc = _get_nc(use_mask)

    in_maps = []
    for b in range(B):
        m = {
            "input_Q": np.ascontiguousarray(input_Q[b]),
            "input_K": np.ascontiguousarray(input_K[b]),
            "input_V": np.ascontiguousarray(input_V[b]),
            "W_Q": np.asarray(W_Q),
            "W_K": np.asarray(W_K),
            "W_V": np.asarray(W_V),
            "W_O": np.asarray(W_O),
        }
        if use_mask:
            madd8 = (8e9 * (np.asarray(attn_mask[b], np.float64) - 1.0)).astype(
                np.float32
            )
            m["madd8"] = np.ascontiguousarray(madd8)
            m["madd8T"] = np.ascontiguousarray(madd8.T)
        in_maps.append(m)

    res = run_bass_kernel_spmd(nc, in_maps, core_ids=list(range(B)))
    out = np.stack([res.results[b]["out"] for b in range(B)], axis=0)
    attn = np.stack([res.results[b]["attn"] for b in range(B)], axis=0)
    return out, attn
